# revision 25
# baseline (speedup 1.0000x reference)
"""Trainium2 Bass kernel for GQA (nn_GQA_28561532518475).

8 cores = 4 batches x 2 kv-head halves.  perm is folded into the weights on
the host (Wq cols -> slot order * scale, Wk/Wv rows by argsort(perm), Wp rows
by perm), so the device kernel is a plain GQA.

v2 changes vs baseline:
- Scores matmuls for the two grouped q-heads (g=0 rows 0:64, g=1 rows 64:128)
  are emitted adjacently so they run CONCURRENTLY via PE row tiling
  (tile_position auto-derived from base_partition) -> ~2x on the scores GEMM.
- Both heads of a kv group stream through the same i-loop, with exp of the
  [128,1024] score tiles split between the Scalar engine (hardware exp) and
  the Vector engine (Schraudolph exp2 bit-trick: one tensor_scalar affine with
  int32 output, bitcast back to fp32) to break the ACT 1-elem/lane/cycle
  bottleneck (~220us if all exp on ACT).
- v projection moving operand padded to 256 cols (fp32r below 256 runs at
  1/4 rate on a warm PE).
- V for all 3 kv heads lives in one [128, PT, NKV, HD+1] tile with a ones
  column so the attention matmul also emits the softmax denominator.
"""

import numpy as np

B, P, C = 4, 2048, 768
H, HK, HD, GS = 12, 6, 64, 2
SCALE = HD ** -0.5
NKV = 3          # kv heads per core
NH = 6           # q heads per core
KT = C // 128    # 6 contraction tiles
PT = P // 128    # 16 row tiles
QB = 1024        # q-block width for attention
NQB = P // QB    # 2

# Schraudolph exp2 constants (bf16 variant): exp(s) ~= bitcast_bf16(
# round_i16(s*log2e*2^7 + ((127 - 0.0564) * 2^7))).  c = -0.0564 zeroes the
# mean relative error of the mantissa-linear-interp over a uniform
# fractional part.
_EXP_S1 = float(1.4426950408889634 * (1 << 7))
_EXP_S2 = float((127 << 7) - 0.0564 * (1 << 7))

_cached_nc = None


def _build_program():
    global _cached_nc
    if _cached_nc is not None:
        return _cached_nc

    import concourse.bass as bass
    import concourse.mybir as mybir
    import concourse.tile as tile
    from concourse import bacc

    fp32 = mybir.dt.float32
    fp32r = mybir.dt.float32r
    bf16 = mybir.dt.bfloat16
    i16 = mybir.dt.int16
    EXP = mybir.ActivationFunctionType.Exp
    MULT = mybir.AluOpType.mult
    ADD = mybir.AluOpType.add

    nc = bacc.Bacc("TRN2", target_bir_lowering=False, debug=False)
    xT = nc.dram_tensor("xT", [C, P], bf16, kind="ExternalInput").ap()
    wq = nc.dram_tensor("wq", [C, 384], bf16, kind="ExternalInput").ap()
    wkd = nc.dram_tensor("wkd", [C, 384], bf16, kind="ExternalInput").ap()
    wv = nc.dram_tensor("wv", [C, 256], bf16, kind="ExternalInput").ap()
    wp = nc.dram_tensor("wp", [384, C], bf16, kind="ExternalInput").ap()
    y = nc.dram_tensor("y", [P, C], fp32, kind="ExternalOutput").ap()
    rcd = nc.dram_tensor("rcd", [NH, P], fp32).ap()
    rcd2 = nc.dram_tensor("rcd2", [NH, P], fp32).ap()

    with tile.TileContext(nc) as tc:
        from contextlib import ExitStack

        with ExitStack() as ctx:
            wpool = ctx.enter_context(tc.tile_pool(name="weights", bufs=1))
            qkvp = ctx.enter_context(tc.tile_pool(name="qkv", bufs=1))
            outp = ctx.enter_context(tc.tile_pool(name="outT", bufs=1))
            epool = ctx.enter_context(tc.tile_pool(name="E", bufs=8))
            nrmp = ctx.enter_context(tc.tile_pool(name="norm", bufs=2))
            ysbp = ctx.enter_context(tc.tile_pool(name="ysb", bufs=3))

            # x kept as 4 column-quarter tiles so consumers start after the
            # first quarter's DMA instead of the full transfer; first quarter
            # and qk weights are queued first so qk_proj(0) unblocks early
            xts = [wpool.tile([128, KT, 512], bf16, name=f"xt{q}") for q in range(4)]
            wv_sb = wpool.tile([128, KT, 256], bf16)
            wqs = [wpool.tile([128, KT, 128], bf16, name=f"wq{m}") for m in range(NKV)]
            wks = [wpool.tile([128, KT, 128], bf16, name=f"wk{m}") for m in range(NKV)]
            wp_sb = wpool.tile([128, 3, C], bf16)
            for kc in range(KT):
                nc.sync.dma_start(
                    xts[0][:, kc, :], xT[kc * 128 : (kc + 1) * 128, 0:512]
                )
            nc.sync.dma_start(
                wqs[0][:], wq[:, 0:128].rearrange("(t p) n -> p t n", p=128)
            )
            nc.sync.dma_start(
                wks[0][:], wkd[:, 0:128].rearrange("(t p) n -> p t n", p=128)
            )
            nc.sync.dma_start(wv_sb[:], wv.rearrange("(t p) n -> p t n", p=128))
            for q in range(1, 4):
                for kc in range(KT):
                    nc.sync.dma_start(
                        xts[q][:, kc, :],
                        xT[kc * 128 : (kc + 1) * 128, q * 512 : (q + 1) * 512],
                    )
            for m in range(1, NKV):
                nc.sync.dma_start(
                    wqs[m][:],
                    wq[:, m * 128 : (m + 1) * 128].rearrange("(t p) n -> p t n", p=128),
                )
                nc.sync.dma_start(
                    wks[m][:],
                    wkd[:, m * 128 : (m + 1) * 128].rearrange("(t p) n -> p t n", p=128),
                )
            nc.sync.dma_start(wp_sb[:], wp.rearrange("(t p) n -> p t n", p=128))

            qts = [qkvp.tile([128, P], bf16, name=f"qt{m}", tag=f"qt{m}") for m in range(NKV)]
            kts = [qkvp.tile([128, P], bf16, name=f"kt{m}", tag=f"kt{m}") for m in range(NKV)]
            # all 3 kv heads' V in one tile: [128, PT, NKV, HD+1], col HD = ones
            vex = qkvp.tile([128, PT, NKV, HD + 1], bf16, name="vex", tag="vex")
            nc.vector.memset(vex[:, :, :, HD], 1.0)
            outTs = [outp.tile([128, P], bf16, name=f"oT{m}", tag=f"oT{m}") for m in range(NKV)]

            expctr = [0]

            with tc.tile_pool(name="mm_ps", bufs=4, space="PSUM") as sps, tc.tile_pool(
                name="o_ps", bufs=1, space="PSUM"
            ) as ops:

                def emit_qk_group(kv, di, nb):
                    w_sb, dest = ((wqs[kv], qts[kv]), (wks[kv], kts[kv]))[di]
                    ps = sps.tile([128, 512], fp32, name="pj", tag="s")
                    for kc in range(KT):
                        nc.tensor.matmul(
                            ps[:],
                            w_sb[:, kc, :],
                            xts[nb][:, kc, :],
                            start=(kc == 0),
                            stop=(kc == KT - 1),
                        )
                    # alternate the psum->sbuf copies between DVE and ACT
                    if (di * 4 + nb) % 2 == 0:
                        nc.vector.tensor_copy(dest[:, nb * 512 : (nb + 1) * 512], ps[:])
                    else:
                        nc.scalar.copy(dest[:, nb * 512 : (nb + 1) * 512], ps[:])

                def qk_proj(kv):
                    # nb-major so the first groups only need the first x
                    # quarter + first weight slices
                    for nb in range(4):
                        for di in range(2):
                            emit_qk_group(kv, di, nb)

                def qk_fillers(kv):
                    # 8 filler units, one qk psum group each
                    for di in range(2):
                        for nb in range(4):
                            yield lambda di=di, nb=nb: emit_qk_group(kv, di, nb)

                def emit_v_group(i):
                    ps = sps.tile([128, 256], fp32, name="vp", tag="s")
                    for kc in range(KT):
                        nc.tensor.matmul(
                            ps[:],
                            xts[i // 4][:, kc, (i % 4) * 128 : (i % 4 + 1) * 128],
                            wv_sb[:, kc, :],
                            start=(kc == 0),
                            stop=(kc == KT - 1),
                        )
                    psv = ps.rearrange("p (h d) -> p h d", h=4)
                    # one strided copy: [128, 3, 64] -> vex[:, i, :, 0:64]
                    nc.scalar.copy(vex[:, i, :, 0:HD], psv[:, 0:NKV, :])

                def emit_out_chunk(mt, nh):
                    ps = sps.tile([128, 384], fp32, name="yp", tag="s")
                    for kf in range(3):
                        nc.tensor.matmul(
                            ps[:],
                            outTs[kf][:, mt * 128 : (mt + 1) * 128],
                            wp_sb[:, kf, nh * 384 : (nh + 1) * 384],
                            start=(kf == 0),
                            stop=(kf == 2),
                        )
                    ysb = ysbp.tile([128, 384], fp32, tag="y")
                    if (mt + nh) % 2 == 0:
                        nc.vector.tensor_copy(ysb[:], ps[:])
                    else:
                        nc.scalar.copy(ysb[:], ps[:])
                    nc.gpsimd.dma_start(
                        y[mt * 128 : (mt + 1) * 128, nh * 384 : (nh + 1) * 384],
                        ysb[:],
                    )

                def emit_exp(e_i16, s_ps, dve_half):
                    if dve_half % 2 == 1:
                        nc.vector.tensor_scalar(
                            e_i16[:], s_ps[:], _EXP_S1, _EXP_S2, MULT, ADD
                        )
                    else:
                        nc.scalar.activation(e_i16.bitcast(bf16), s_ps[:], EXP)

                def emit_av(kv, obs, i, es, nbs):
                    for nbi, (off, w) in enumerate(nbs):
                        for g in range(GS):
                            nc.tensor.matmul(
                                obs[g][:, off : off + w],
                                vex[:, i, kv, :],
                                es[(g, nbi)].bitcast(bf16)[:],
                                start=(i == 0),
                                stop=(i == PT - 1),
                            )

                def attention_block(kv, q0, W, fillers=(), last=False):
                    fillers = iter(fillers)
                    nbs = []
                    off = 0
                    while off < W:
                        w = min(512, W - off)
                        nbs.append((off, w))
                        off += w
                    obs = [
                        ops.tile([HD + 1, W], fp32, name=f"ob{g}", tag=f"ob{g}")
                        for g in range(GS)
                    ]
                    prev = None
                    for i in range(PT):
                        # independent score tiles per (head, 512-chunk) on
                        # rotating psum slots so the next iteration's scores
                        # never wait on this iteration's exps
                        sts = {}
                        for nbi, (off, w) in enumerate(nbs):
                            for g in range(GS):
                                gp = slice(g * 64, (g + 1) * 64)
                                s = sps.tile(
                                    [128, w], fp32, name=f"sc{g}{nbi}", tag="s"
                                )
                                sts[(g, nbi)] = s
                                nc.tensor.matmul(
                                    s[:],
                                    kts[kv][gp, i * 128 : (i + 1) * 128],
                                    qts[kv][gp, q0 + off : q0 + off + w],
                                    start=True,
                                    stop=True,
                                )
                        es = {}
                        for (g, nbi), s in sts.items():
                            e = epool.tile([128, nbs[nbi][1]], i16, tag="e")
                            # alternate ACT/DVE so each head stays ~50%
                            # Schraudolph
                            emit_exp(e, s, dve_half=((g + nbi + i) % 2))
                            es[(g, nbi)] = e
                        if prev is not None:
                            emit_av(kv, obs, prev[0], prev[1], nbs)
                        prev = (i, es)
                        # interleave an independent matmul group as PE filler
                        # to keep the HAM clock warm
                        f = next(fillers, None)
                        if f is not None:
                            f()
                    emit_av(kv, obs, prev[0], prev[1], nbs)
                    # drain unconsumed fillers -- they must land before the
                    # next block (e.g. qk projections feeding it)
                    for f in fillers:
                        if f is not None:
                            f()
                    # normalize: free the ob psum quickly via an ACT/DVE copy
                    # to SBUF, then softmax sums -> DRAM -> lane-spread
                    # reciprocal -> DRAM -> partition-broadcast read, and a
                    # GpSimd multiply (DVE on the last block for latency)
                    WL = W // 128
                    for g in range(GS):
                        ob = obs[g]
                        h = 2 * kv + g
                        osb = nrmp.tile([HD + 1, W], fp32, tag=f"osb{g}")
                        if g == 0:
                            nc.scalar.copy(osb[:], ob[:])
                        else:
                            nc.vector.tensor_copy(osb[:], ob[:])
                        nc.sync.dma_start(
                            rcd[h : h + 1, q0 : q0 + W], osb[HD : HD + 1, :]
                        )
                        rr = nrmp.tile([128, WL], fp32, tag="rr")
                        lanes = bass.AP(
                            tensor=rcd.tensor,
                            offset=h * P + q0,
                            ap=[[WL, 128], [1, WL]],
                        )
                        nc.sync.dma_start(rr[:], lanes)
                        rr2 = nrmp.tile([128, WL], fp32, tag="rr2")
                        nc.vector.reciprocal(rr2[:], rr[:])
                        lanes2 = bass.AP(
                            tensor=rcd2.tensor,
                            offset=h * P + q0,
                            ap=[[WL, 128], [1, WL]],
                        )
                        nc.sync.dma_start(lanes2, rr2[:])
                        rb = nrmp.tile([HD, W], fp32, tag="rb")
                        bcast = bass.AP(
                            tensor=rcd2.tensor, offset=h * P + q0, ap=[[0, HD], [1, W]]
                        )
                        nc.gpsimd.dma_start(rb[:], bcast)
                        mul_eng = nc.vector if last else nc.gpsimd
                        if g == 0:
                            mul_eng.tensor_mul(
                                outTs[kv][0:HD, q0 : q0 + W], osb[0:HD, :], rb[:]
                            )
                        else:
                            sc2 = nrmp.tile([HD, W], bf16, tag="sc2")
                            mul_eng.tensor_mul(sc2[:], osb[0:HD, :], rb[:])
                            nc.sync.dma_start(
                                outTs[kv][HD:128, q0 : q0 + W], sc2[:]
                            )

                import itertools

                def v_fillers():
                    # 2 v groups per filler slot, staying >=2 ahead of the
                    # consuming AV(kv=0) i-loop
                    for i0 in range(4, PT, 2):
                        yield lambda i0=i0: (emit_v_group(i0), emit_v_group(i0 + 1))

                def out_units(mts):
                    for mt in mts:
                        for nh in range(2):
                            yield lambda mt=mt, nh=nh: emit_out_chunk(mt, nh)

                def weave(units, gap=1, lead=0):
                    return itertools.chain(
                        itertools.repeat(None, lead),
                        itertools.chain.from_iterable(
                            (f,) + (None,) * gap for f in units
                        ),
                    )

                qk_proj(0)
                for i in range(4):
                    emit_v_group(i)

                # kv-major schedule; kv=2's second half is split (768, 256)
                # so the final serial normalize + out_proj tail covers only
                # 256 q columns, with out_proj chunks of completed q-ranges
                # filling the last two blocks' exp bubbles.
                SPLITS = {
                    0: [(0, 1024), (1024, 1024)],
                    1: [(0, 1024), (1024, 1024)],
                    2: [(0, 1024), (1024, 768), (1792, 256)],
                }
                qk2 = list(qk_fillers(2))
                FILL = {
                    (0, 0): weave(v_fillers()),
                    (0, 1): weave(qk_fillers(1)),
                    (1, 0): weave(qk2[:4], gap=3),
                    (1, 1): weave(qk2[4:], gap=3),
                    (2, 1): itertools.chain(
                        itertools.repeat(None, 3), out_units(range(0, 6))
                    ),
                    (2, 2): itertools.chain(
                        itertools.repeat(None, 1), out_units(range(6, 13))
                    ),
                }
                for kv in range(NKV):
                    splits = SPLITS[kv]
                    for bi, (q0, W) in enumerate(splits):
                        attention_block(
                            kv,
                            q0,
                            W,
                            FILL.get((kv, bi), ()),
                            last=(kv == 2 and bi == len(splits) - 1),
                        )
                for mt in (13, 14, 15):
                    for nh in range(2):
                        emit_out_chunk(mt, nh)

    nc.compile()
    _cached_nc = nc
    return nc


def _make_in_maps(x, Wq, Wk, Wv, Wp, perm):
    from ml_dtypes import bfloat16

    inv = np.argsort(perm)
    Wq_f = np.ascontiguousarray(
        Wq.reshape(C, H, HD)[:, perm, :].reshape(C, C) * SCALE
    )
    Wk_f = np.ascontiguousarray(Wk.reshape(H, HD, HK * HD)[inv].reshape(C, HK * HD))
    Wv_f = np.ascontiguousarray(Wv.reshape(H, HD, HK * HD)[inv].reshape(C, HK * HD))
    Wp_f = np.ascontiguousarray(Wp.reshape(H, HD, C)[perm].reshape(C, C))

    in_maps = []
    for core in range(8):
        b, half = core // 2, core % 2
        wk_half = Wk_f[:, half * 192 : (half + 1) * 192].reshape(C, NKV, 1, HD)
        wkd = np.ascontiguousarray(
            np.broadcast_to(wk_half, (C, NKV, 2, HD)).reshape(C, 384)
        )
        wv_pad = np.zeros((C, 256), bfloat16)
        wv_pad[:, :192] = Wv_f[:, half * 192 : (half + 1) * 192]
        in_maps.append(
            {
                "xT": np.ascontiguousarray(x[b].T.astype(bfloat16)),
                "wq": np.ascontiguousarray(
                    Wq_f[:, half * 384 : (half + 1) * 384].astype(bfloat16)
                ),
                "wkd": wkd.astype(bfloat16),
                "wv": wv_pad,
                "wp": np.ascontiguousarray(
                    Wp_f[half * 384 : (half + 1) * 384, :].astype(bfloat16)
                ),
            }
        )
    return in_maps


def kernel(x, Wq, Wk, Wv, Wp, bp, bass_run_kwargs=None, **_unused):
    perm = _unused.pop("perm")
    from concourse.bass_utils import run_bass_kernel_spmd

    x = np.asarray(x, np.float32)
    nc = _build_program()
    in_maps = _make_in_maps(
        x,
        np.asarray(Wq, np.float32),
        np.asarray(Wk, np.float32),
        np.asarray(Wv, np.float32),
        np.asarray(Wp, np.float32),
        np.asarray(perm),
    )
    res = run_bass_kernel_spmd(
        nc, in_maps, core_ids=list(range(8)), **(bass_run_kwargs or {})
    )
    bp = np.asarray(bp, np.float32)
    y = np.empty((B, P, C), np.float32)
    for b in range(B):
        y[b] = res.results[2 * b]["y"] + res.results[2 * b + 1]["y"] + bp
    if bass_run_kwargs:
        kernel.last_results = res
    return y


# revision 26
# speedup vs baseline: 1.0907x; 1.0907x over previous
"""Trainium2 Bass kernel for GQA (nn_GQA_28561532518475).

8 cores = 4 batches x 2 kv-head halves.  perm is folded into the weights on
the host (Wq cols -> slot order * scale, Wk/Wv rows by argsort(perm), Wp rows
by perm), so the device kernel is a plain GQA.

v2 changes vs baseline:
- Scores matmuls for the two grouped q-heads (g=0 rows 0:64, g=1 rows 64:128)
  are emitted adjacently so they run CONCURRENTLY via PE row tiling
  (tile_position auto-derived from base_partition) -> ~2x on the scores GEMM.
- Both heads of a kv group stream through the same i-loop, with exp of the
  [128,1024] score tiles split between the Scalar engine (hardware exp) and
  the Vector engine (Schraudolph exp2 bit-trick: one tensor_scalar affine with
  int32 output, bitcast back to fp32) to break the ACT 1-elem/lane/cycle
  bottleneck (~220us if all exp on ACT).
- v projection moving operand padded to 256 cols (fp32r below 256 runs at
  1/4 rate on a warm PE).
- V for all 3 kv heads lives in one [128, PT, NKV, HD+1] tile with a ones
  column so the attention matmul also emits the softmax denominator.
"""

import numpy as np

B, P, C = 4, 2048, 768
H, HK, HD, GS = 12, 6, 64, 2
SCALE = HD ** -0.5
NKV = 3          # kv heads per core
NH = 6           # q heads per core
KT = C // 128    # 6 contraction tiles
PT = P // 128    # 16 row tiles
QB = 1024        # q-block width for attention
NQB = P // QB    # 2

# Schraudolph exp2 constants (bf16 variant): exp(s) ~= bitcast_bf16(
# round_i16(s*log2e*2^7 + ((127 - 0.0564) * 2^7))).  c = -0.0564 zeroes the
# mean relative error of the mantissa-linear-interp over a uniform
# fractional part.
_EXP_S1 = float(1.4426950408889634 * (1 << 7))
_EXP_S2 = float((127 << 7) - 0.0564 * (1 << 7))

_cached_nc = None


def _build_program():
    global _cached_nc
    if _cached_nc is not None:
        return _cached_nc

    import concourse.bass as bass
    import concourse.mybir as mybir
    import concourse.tile as tile
    from concourse import bacc

    fp32 = mybir.dt.float32
    fp32r = mybir.dt.float32r
    bf16 = mybir.dt.bfloat16
    i16 = mybir.dt.int16
    EXP = mybir.ActivationFunctionType.Exp
    MULT = mybir.AluOpType.mult
    ADD = mybir.AluOpType.add

    nc = bacc.Bacc("TRN2", target_bir_lowering=False, debug=False)
    xT = nc.dram_tensor("xT", [C, P], bf16, kind="ExternalInput").ap()
    wq = nc.dram_tensor("wq", [C, 384], bf16, kind="ExternalInput").ap()
    wkd = nc.dram_tensor("wkd", [C, 384], bf16, kind="ExternalInput").ap()
    wv = nc.dram_tensor("wv", [C, 256], bf16, kind="ExternalInput").ap()
    wp = nc.dram_tensor("wp", [384, C], bf16, kind="ExternalInput").ap()
    y = nc.dram_tensor("y", [P, C], fp32, kind="ExternalOutput").ap()
    rcd = nc.dram_tensor("rcd", [NH, P], fp32).ap()
    rcd2 = nc.dram_tensor("rcd2", [NH, P], fp32).ap()

    with tile.TileContext(nc) as tc:
        from contextlib import ExitStack

        with ExitStack() as ctx:
            wpool = ctx.enter_context(tc.tile_pool(name="weights", bufs=1))
            qkvp = ctx.enter_context(tc.tile_pool(name="qkv", bufs=1))
            outp = ctx.enter_context(tc.tile_pool(name="outT", bufs=1))
            epool = ctx.enter_context(tc.tile_pool(name="E", bufs=8))
            nrmp = ctx.enter_context(tc.tile_pool(name="norm", bufs=2))
            ysbp = ctx.enter_context(tc.tile_pool(name="ysb", bufs=3))

            # x kept as 4 column-quarter tiles so consumers start after the
            # first quarter's DMA instead of the full transfer; first quarter
            # and qk weights are queued first so qk_proj(0) unblocks early
            xts = [wpool.tile([128, KT, 512], bf16, name=f"xt{q}") for q in range(4)]
            wv_sb = wpool.tile([128, KT, 256], bf16)
            wqs = [wpool.tile([128, KT, 128], bf16, name=f"wq{m}") for m in range(NKV)]
            wks = [wpool.tile([128, KT, 128], bf16, name=f"wk{m}") for m in range(NKV)]
            wp_sb = wpool.tile([128, 3, C], bf16)
            for kc in range(KT):
                nc.sync.dma_start(
                    xts[0][:, kc, :], xT[kc * 128 : (kc + 1) * 128, 0:512]
                )
            nc.sync.dma_start(
                wqs[0][:], wq[:, 0:128].rearrange("(t p) n -> p t n", p=128)
            )
            nc.sync.dma_start(
                wks[0][:], wkd[:, 0:128].rearrange("(t p) n -> p t n", p=128)
            )
            nc.sync.dma_start(wv_sb[:], wv.rearrange("(t p) n -> p t n", p=128))
            for q in range(1, 4):
                for kc in range(KT):
                    nc.sync.dma_start(
                        xts[q][:, kc, :],
                        xT[kc * 128 : (kc + 1) * 128, q * 512 : (q + 1) * 512],
                    )
            for m in range(1, NKV):
                nc.sync.dma_start(
                    wqs[m][:],
                    wq[:, m * 128 : (m + 1) * 128].rearrange("(t p) n -> p t n", p=128),
                )
                nc.sync.dma_start(
                    wks[m][:],
                    wkd[:, m * 128 : (m + 1) * 128].rearrange("(t p) n -> p t n", p=128),
                )
            nc.sync.dma_start(wp_sb[:], wp.rearrange("(t p) n -> p t n", p=128))

            qts = [qkvp.tile([128, P], bf16, name=f"qt{m}", tag=f"qt{m}") for m in range(NKV)]
            kts = [qkvp.tile([128, P], bf16, name=f"kt{m}", tag=f"kt{m}") for m in range(NKV)]
            # all 3 kv heads' V in one tile: [128, PT, NKV, HD+1], col HD = ones
            vex = qkvp.tile([128, PT, NKV, HD + 1], bf16, name="vex", tag="vex")
            nc.vector.memset(vex[:, :, :, HD], 1.0)
            outTs = [outp.tile([128, P], bf16, name=f"oT{m}", tag=f"oT{m}") for m in range(NKV)]

            expctr = [0]

            with tc.tile_pool(name="mm_ps", bufs=4, space="PSUM") as sps, tc.tile_pool(
                name="o_ps", bufs=1, space="PSUM"
            ) as ops:

                def emit_qk_group(kv, di, nb):
                    w_sb, dest = ((wqs[kv], qts[kv]), (wks[kv], kts[kv]))[di]
                    ps = sps.tile([128, 512], fp32, name="pj", tag="s")
                    for kc in range(KT):
                        nc.tensor.matmul(
                            ps[:],
                            w_sb[:, kc, :],
                            xts[nb][:, kc, :],
                            start=(kc == 0),
                            stop=(kc == KT - 1),
                        )
                    # alternate the psum->sbuf copies between DVE and ACT
                    if (di * 4 + nb) % 2 == 0:
                        nc.vector.tensor_copy(dest[:, nb * 512 : (nb + 1) * 512], ps[:])
                    else:
                        nc.scalar.copy(dest[:, nb * 512 : (nb + 1) * 512], ps[:])

                def qk_proj(kv):
                    # nb-major so the first groups only need the first x
                    # quarter + first weight slices
                    for nb in range(4):
                        for di in range(2):
                            emit_qk_group(kv, di, nb)

                def qk_fillers(kv):
                    # 8 filler units, one qk psum group each
                    for di in range(2):
                        for nb in range(4):
                            yield lambda di=di, nb=nb: emit_qk_group(kv, di, nb)

                def emit_v_group(i):
                    ps = sps.tile([128, 256], fp32, name="vp", tag="s")
                    for kc in range(KT):
                        nc.tensor.matmul(
                            ps[:],
                            xts[i // 4][:, kc, (i % 4) * 128 : (i % 4 + 1) * 128],
                            wv_sb[:, kc, :],
                            start=(kc == 0),
                            stop=(kc == KT - 1),
                        )
                    psv = ps.rearrange("p (h d) -> p h d", h=4)
                    # one strided copy: [128, 3, 64] -> vex[:, i, :, 0:64]
                    nc.scalar.copy(vex[:, i, :, 0:HD], psv[:, 0:NKV, :])

                def emit_out_chunk(mt, nh):
                    ps = sps.tile([128, 384], fp32, name="yp", tag="s")
                    for kf in range(3):
                        nc.tensor.matmul(
                            ps[:],
                            outTs[kf][:, mt * 128 : (mt + 1) * 128],
                            wp_sb[:, kf, nh * 384 : (nh + 1) * 384],
                            start=(kf == 0),
                            stop=(kf == 2),
                        )
                    ysb = ysbp.tile([128, 384], fp32, tag="y")
                    if (mt + nh) % 2 == 0:
                        nc.vector.tensor_copy(ysb[:], ps[:])
                    else:
                        nc.scalar.copy(ysb[:], ps[:])
                    nc.gpsimd.dma_start(
                        y[mt * 128 : (mt + 1) * 128, nh * 384 : (nh + 1) * 384],
                        ysb[:],
                    )

                def emit_exp(e_i16, s_ps, dve_half):
                    if dve_half % 2 == 1:
                        nc.vector.tensor_scalar(
                            e_i16[:], s_ps[:], _EXP_S1, _EXP_S2, MULT, ADD
                        )
                    else:
                        nc.scalar.activation(e_i16.bitcast(bf16), s_ps[:], EXP)

                def emit_av(kv, obs, i, es, nbs):
                    for nbi, (off, w) in enumerate(nbs):
                        for g in range(GS):
                            nc.tensor.matmul(
                                obs[g][:, off : off + w],
                                vex[:, i, kv, :],
                                es[(g, nbi)].bitcast(bf16)[:],
                                start=(i == 0),
                                stop=(i == PT - 1),
                            )

                def attention_block(kv, q0, W, fillers=(), last=False):
                    fillers = iter(fillers)
                    nbs = []
                    off = 0
                    while off < W:
                        w = min(512, W - off)
                        nbs.append((off, w))
                        off += w
                    obs = [
                        ops.tile([HD + 1, W], fp32, name=f"ob{g}", tag=f"ob{g}")
                        for g in range(GS)
                    ]
                    prev = None
                    for i in range(PT):
                        # independent score tiles per (head, 512-chunk) on
                        # rotating psum slots so the next iteration's scores
                        # never wait on this iteration's exps
                        sts = {}
                        for nbi, (off, w) in enumerate(nbs):
                            for g in range(GS):
                                gp = slice(g * 64, (g + 1) * 64)
                                s = sps.tile(
                                    [128, w], fp32, name=f"sc{g}{nbi}", tag="s"
                                )
                                sts[(g, nbi)] = s
                                nc.tensor.matmul(
                                    s[:],
                                    kts[kv][gp, i * 128 : (i + 1) * 128],
                                    qts[kv][gp, q0 + off : q0 + off + w],
                                    start=True,
                                    stop=True,
                                )
                        es = {}
                        for (g, nbi), s in sts.items():
                            e = epool.tile([128, nbs[nbi][1]], i16, tag="e")
                            # alternate ACT/DVE so each head stays ~50%
                            # Schraudolph
                            emit_exp(e, s, dve_half=((g + nbi + i) % 2))
                            es[(g, nbi)] = e
                        if prev is not None:
                            emit_av(kv, obs, prev[0], prev[1], nbs)
                        prev = (i, es)
                        # interleave an independent matmul group as PE filler
                        # to keep the HAM clock warm
                        f = next(fillers, None)
                        if f is not None:
                            f()
                    emit_av(kv, obs, prev[0], prev[1], nbs)
                    # drain unconsumed fillers -- they must land before the
                    # next block (e.g. qk projections feeding it)
                    for f in fillers:
                        if f is not None:
                            f()
                    # normalize: free the ob psum quickly via an ACT/DVE copy
                    # to SBUF, then softmax sums -> DRAM -> lane-spread
                    # reciprocal -> DRAM -> partition-broadcast read, and a
                    # GpSimd multiply (DVE on the last block for latency)
                    WL = W // 128
                    for g in range(GS):
                        ob = obs[g]
                        h = 2 * kv + g
                        osb = nrmp.tile([HD + 1, W], fp32, tag=f"osb{g}")
                        if g == 0:
                            nc.scalar.copy(osb[:], ob[:])
                        else:
                            nc.vector.tensor_copy(osb[:], ob[:])
                        nc.sync.dma_start(
                            rcd[h : h + 1, q0 : q0 + W], osb[HD : HD + 1, :]
                        )
                        rr = nrmp.tile([128, WL], fp32, tag="rr")
                        lanes = bass.AP(
                            tensor=rcd.tensor,
                            offset=h * P + q0,
                            ap=[[WL, 128], [1, WL]],
                        )
                        nc.sync.dma_start(rr[:], lanes)
                        rr2 = nrmp.tile([128, WL], fp32, tag="rr2")
                        nc.vector.reciprocal(rr2[:], rr[:])
                        lanes2 = bass.AP(
                            tensor=rcd2.tensor,
                            offset=h * P + q0,
                            ap=[[WL, 128], [1, WL]],
                        )
                        nc.sync.dma_start(lanes2, rr2[:])
                        rb = nrmp.tile([HD, W], fp32, tag="rb")
                        bcast = bass.AP(
                            tensor=rcd2.tensor, offset=h * P + q0, ap=[[0, HD], [1, W]]
                        )
                        nc.gpsimd.dma_start(rb[:], bcast)
                        mul_eng = nc.vector if last else nc.gpsimd
                        if g == 0:
                            mul_eng.tensor_mul(
                                outTs[kv][0:HD, q0 : q0 + W], osb[0:HD, :], rb[:]
                            )
                        else:
                            sc2 = nrmp.tile([HD, W], bf16, tag="sc2")
                            mul_eng.tensor_mul(sc2[:], osb[0:HD, :], rb[:])
                            nc.sync.dma_start(
                                outTs[kv][HD:128, q0 : q0 + W], sc2[:]
                            )

                import itertools

                def v_fillers():
                    # 2 v groups per filler slot, staying >=2 ahead of the
                    # consuming AV(kv=0) i-loop
                    for i0 in range(4, PT, 2):
                        yield lambda i0=i0: (emit_v_group(i0), emit_v_group(i0 + 1))

                def out_units(mts):
                    for mt in mts:
                        for nh in range(2):
                            yield lambda mt=mt, nh=nh: emit_out_chunk(mt, nh)

                def weave(units, gap=1, lead=0):
                    return itertools.chain(
                        itertools.repeat(None, lead),
                        itertools.chain.from_iterable(
                            (f,) + (None,) * gap for f in units
                        ),
                    )

                qk_proj(0)
                for i in range(4):
                    emit_v_group(i)

                # kv-major schedule; kv=2's second half is split (768, 256)
                # so the final serial normalize + out_proj tail covers only
                # 256 q columns, with out_proj chunks of completed q-ranges
                # filling the last two blocks' exp bubbles.
                SPLITS = {
                    0: [(0, 1024), (1024, 1024)],
                    1: [(0, 1024), (1024, 1024)],
                    2: [(0, 1024), (1024, 1024)],
                }
                qk2 = list(qk_fillers(2))
                FILL = {
                    (0, 0): weave(v_fillers()),
                    (0, 1): weave(qk_fillers(1)),
                    (1, 0): weave(qk2[:4], gap=3),
                    (1, 1): weave(qk2[4:], gap=3),
                    (2, 1): itertools.chain(
                        itertools.repeat(None, 4), out_units(range(0, 8))
                    ),
                }
                for kv in range(NKV):
                    splits = SPLITS[kv]
                    for bi, (q0, W) in enumerate(splits):
                        attention_block(
                            kv,
                            q0,
                            W,
                            FILL.get((kv, bi), ()),
                            last=(kv == 2 and bi == len(splits) - 1),
                        )
                for mt in range(8, PT):
                    for nh in range(2):
                        emit_out_chunk(mt, nh)

    nc.compile()
    _cached_nc = nc
    return nc


def _make_in_maps(x, Wq, Wk, Wv, Wp, perm):
    from ml_dtypes import bfloat16

    inv = np.argsort(perm)
    Wq_f = np.ascontiguousarray(
        Wq.reshape(C, H, HD)[:, perm, :].reshape(C, C) * SCALE
    )
    Wk_f = np.ascontiguousarray(Wk.reshape(H, HD, HK * HD)[inv].reshape(C, HK * HD))
    Wv_f = np.ascontiguousarray(Wv.reshape(H, HD, HK * HD)[inv].reshape(C, HK * HD))
    Wp_f = np.ascontiguousarray(Wp.reshape(H, HD, C)[perm].reshape(C, C))

    in_maps = []
    for core in range(8):
        b, half = core // 2, core % 2
        wk_half = Wk_f[:, half * 192 : (half + 1) * 192].reshape(C, NKV, 1, HD)
        wkd = np.ascontiguousarray(
            np.broadcast_to(wk_half, (C, NKV, 2, HD)).reshape(C, 384)
        )
        wv_pad = np.zeros((C, 256), bfloat16)
        wv_pad[:, :192] = Wv_f[:, half * 192 : (half + 1) * 192]
        in_maps.append(
            {
                "xT": np.ascontiguousarray(x[b].T.astype(bfloat16)),
                "wq": np.ascontiguousarray(
                    Wq_f[:, half * 384 : (half + 1) * 384].astype(bfloat16)
                ),
                "wkd": wkd.astype(bfloat16),
                "wv": wv_pad,
                "wp": np.ascontiguousarray(
                    Wp_f[half * 384 : (half + 1) * 384, :].astype(bfloat16)
                ),
            }
        )
    return in_maps


def kernel(x, Wq, Wk, Wv, Wp, bp, bass_run_kwargs=None, **_unused):
    perm = _unused.pop("perm")
    from concourse.bass_utils import run_bass_kernel_spmd

    x = np.asarray(x, np.float32)
    nc = _build_program()
    in_maps = _make_in_maps(
        x,
        np.asarray(Wq, np.float32),
        np.asarray(Wk, np.float32),
        np.asarray(Wv, np.float32),
        np.asarray(Wp, np.float32),
        np.asarray(perm),
    )
    res = run_bass_kernel_spmd(
        nc, in_maps, core_ids=list(range(8)), **(bass_run_kwargs or {})
    )
    bp = np.asarray(bp, np.float32)
    y = np.empty((B, P, C), np.float32)
    for b in range(B):
        y[b] = res.results[2 * b]["y"] + res.results[2 * b + 1]["y"] + bp
    if bass_run_kwargs:
        kernel.last_results = res
    return y
